# revision 38
# baseline (speedup 1.0000x reference)
"""Causal single-head attention on 8 TRN2 NeuronCores (v2).

Problem: x:(S=4096, B=4, E=5) f32; Wk/Wq/Wv:(5,64), bk/bq/bv:(64,).
  K/Q/V = x@W + b per batch; scores = K.Q^T/8 (keys i, queries j), causal
  (key i attends query j iff i <= j), softmax over keys per query, out =
  sum_i V[i]*P[i,j] -> (S, B, 64).

Sharding: 8 cores = 4 batches x 2 query-stripe parities. Parity 0 takes
query tiles at offsets {0,1024,2048,3072}, parity 1 {512,1536,2560,3584}.
One SPMD graph; per-core differences are pure input data.

Key algebraic tricks (all host-precomputed):
  - scores = X6 @ M6 @ X6^T where X6 = [x | 1] (S,6) and M6 (6,6) folds
    Wk, Wq, both biases and the 1/sqrt(64) scale. G = X6 @ M6 is computed
    on host, so mm1 per 128-key block is a K=6 contraction:
    lhsT = G^T block (6,128), rhs = X6^T queries (6,512).
  - V6 = [x@Wv + bv | 1] (S,65); mm2 accumulates O^T = sum_blocks
    V6_blk^T @ P_blk directly into one PSUM bank per query slot. Column
    64 (the ones column) accumulates the softmax denominator.

Schedule per core: 4 query slots x 512; per slot, key blocks (128 keys)
grouped into "triad" units of <=3 blocks. Per unit: 3 row-tiled mm1
matmuls (tile groups at partitions 0/32/64 run concurrently), one Exp
activation over the whole [128, 3*512] PSUM unit (amortizes ACT
instruction overhead), DVE causal-mask multiplies on diagonal blocks,
then 3 accumulating mm2 matmuls. Slot epilogue: PE transpose + DVE
reciprocal/mul + DMA out. PSUM: st 2x3 banks, ot 1, tr 1 = 8.
No max-subtraction (scores are O(1), exp is safe).
"""

import sys
from contextlib import ExitStack

import ml_dtypes
import numpy as np

for _p in ("/opt/trn_rl_repo", "/opt/pypackages"):
    if _p not in sys.path:
        sys.path.append(_p)

import concourse.bass as bass
import concourse.tile as tile
from concourse import bacc, mybir

F32 = mybir.dt.float32
F16 = mybir.dt.float16
I16 = mybir.dt.int16
BF16 = mybir.dt.bfloat16

S, B, E, NE = 4096, 4, 5, 64
N_CORES = 8
JT = 512            # query tile width
NSLOT = 4
FCNT = (4, 12, 20, 28)   # static full-block count per slot (parity max)
JOS_BY_PARITY = ((0, 1024, 2048, 3072), (512, 1536, 2560, 3584))
UNIT_CAP = 3        # key blocks per unit (3 PSUM banks per st buffer)
SLOT_ORDER = (3, 2, 1, 0)   # biggest first: shorter tail

# ---- static unit tables (parity-independent structure) ----
# entry = ('F', f_idx) or ('D', d); per slot: FCNT full blocks + 4 diag.
SLOT_UNITS = []
for _t in range(NSLOT):
    _L = [("F", g) for g in range(FCNT[_t])] + [("D", d) for d in range(4)]
    SLOT_UNITS.append([_L[i : i + UNIT_CAP] for i in range(0, len(_L), UNIT_CAP)])
N_UNITS = sum(len(u) for u in SLOT_UNITS)            # 28
N_BLOCKS = sum(len(un) for u in SLOT_UNITS for un in u)  # 80
# flat offsets in PROCESSING order (slot 3 first) so chunked DMAs can
# deliver the first-processed units' data first:
# UNIT_OFS[t][u] = unit index into g4 columns; BLK_OFS[t][u] = index of
# the unit's first block into x6v blocks.
UNIT_OFS = [[0] * len(SLOT_UNITS[_t]) for _t in range(NSLOT)]
BLK_OFS = [[0] * len(SLOT_UNITS[_t]) for _t in range(NSLOT)]
_uc, _bc = 0, 0
for _t in SLOT_ORDER:
    for _u, _un in enumerate(SLOT_UNITS[_t]):
        UNIT_OFS[_t][_u] = _uc
        BLK_OFS[_t][_u] = _bc
        _uc += 1
        _bc += len(_un)

# units whose exp runs on the vector engine via the Schraudolph fp16
# bit trick (all-F units only; spread across the timeline).
SCHR_UNITS = {(3, 2), (3, 4), (3, 6), (3, 8), (2, 2), (2, 4), (1, 2), (0, 0)}
A16 = 1024.0 / float(np.log(2.0))        # 1477.3197
B16 = 15360.0 - 45.0                     # fp16 exponent bias - spline tweak

# last unit index that contributes a block at position p (per slot), for
# mm2 PSUM accumulation stop flags.
LAST_U_AT_P = []
for _t in range(NSLOT):
    LAST_U_AT_P.append(
        [max(_u for _u, _un in enumerate(SLOT_UNITS[_t]) if len(_un) > _p)
         for _p in range(UNIT_CAP)]
    )

_NC_CACHE = {}


def build_graph():
    nc = bacc.Bacc("TRN2", target_bir_lowering=False, debug=False)

    g4 = nc.declare_dram_parameter("g4", [128, N_UNITS * 128], BF16, isOutput=False)
    xq4 = nc.declare_dram_parameter("xq4", [128, NSLOT * JT], BF16, isOutput=False)
    x6v = nc.declare_dram_parameter("x6v", [128, N_BLOCKS * (NE + 1)], BF16,
                                    isOutput=False)
    out = nc.declare_dram_parameter("out", [NSLOT * JT, NE], F32, isOutput=True)

    with tile.TileContext(nc) as tc, ExitStack() as ctx:
        consts = ctx.enter_context(tc.tile_pool(name="consts", bufs=1))
        psum = ctx.enter_context(tc.tile_pool(name="psum", bufs=2, space="PSUM"))
        sb = ctx.enter_context(tc.tile_pool(name="sb", bufs=2))

        xq4_sb = consts.tile([128, NSLOT * JT], BF16)
        g4_sb = consts.tile([128, N_UNITS * 128], BF16)
        x6v_sb = consts.tile([128, N_BLOCKS * (NE + 1)], BF16)
        # Chunked input DMAs in processing order: the first-processed
        # units' stationaries land first so mm1 of unit 0 starts ~1us in.
        t0 = SLOT_ORDER[0]
        nc.sync.dma_start(
            out=xq4_sb[:, t0 * JT : (t0 + 1) * JT],
            in_=xq4[:, t0 * JT : (t0 + 1) * JT],
        )
        G4C0, BLKC0 = 4 * 128, 12 * (NE + 1)
        nc.sync.dma_start(out=g4_sb[:, 0:G4C0], in_=g4[:, 0:G4C0])
        nc.sync.dma_start(out=x6v_sb[:, 0:BLKC0], in_=x6v[:, 0:BLKC0])
        nc.sync.dma_start(out=g4_sb[:, G4C0:], in_=g4[:, G4C0:])
        nc.sync.dma_start(out=xq4_sb[:, 0 : t0 * JT], in_=xq4[:, 0 : t0 * JT])
        nc.sync.dma_start(out=x6v_sb[:, BLKC0:], in_=x6v[:, BLKC0:])

        # PE warmup: one contiguous ~4.5us burst of matmuls right after
        # the first DMA chunk lands, covering a full free-running HAM
        # window so the PE clock un-gates to 8/8 before the main loop.
        warm_ps = psum.tile([128, JT], F32, tag="ot", bufs=2, name="warm_ps")
        for _ in range(8):
            nc.tensor.matmul(
                warm_ps[:, :],
                xq4_sb[0:6, t0 * JT : t0 * JT + 128],
                xq4_sb[0:6, t0 * JT : (t0 + 1) * JT],
                start=True, stop=True, skip_group_check=True,
            )

        # identity for PE transpose
        ident_f = consts.tile([128, 128], F32)
        from concourse.masks import make_identity

        make_identity(nc, ident_f[:])

        # diagonal causal masks, core-invariant: mask_d[p, q] = (p + 128d <= q)
        masks_f = consts.tile([128, 4 * JT], F32)
        nc.gpsimd.memset(masks_f[:], 1.0)
        for d in range(4):
            nc.gpsimd.affine_select(
                out=masks_f[:, d * JT : (d + 1) * JT],
                in_=masks_f[:, d * JT : (d + 1) * JT],
                compare_op=mybir.AluOpType.is_ge,
                fill=0.0,
                base=-128 * d,
                pattern=[[1, JT]],
                channel_multiplier=-1,
            )
        masks_sb = consts.tile([128, 4 * JT], BF16)
        nc.vector.tensor_copy(masks_sb[:], masks_f[:])

        # ---- main pipeline ----
        # front(k): mm1 unit k + Exp + masks.  back(k): mm2 accumulate
        # (+ slot epilogue when k closes a slot), emitted after front(k+1)
        # so the PE never waits on ACT(k) before starting mm1(k+1).
        fronts = []
        for t in SLOT_ORDER:
            for u, unit in enumerate(SLOT_UNITS[t]):
                fronts.append((t, u, unit))

        ot_tiles = {}

        def emit_front(t, u, unit):
            size = len(unit)
            st = psum.tile([128, UNIT_CAP * JT], F32, tag="st", bufs=2)
            gcol = UNIT_OFS[t][u] * 128
            for p in range(size):
                nc.tensor.matmul(
                    st[:, p * JT : (p + 1) * JT],
                    g4_sb[32 * p : 32 * p + 6, gcol : gcol + 128],
                    xq4_sb[32 * p : 32 * p + 6, t * JT : (t + 1) * JT],
                    start=True,
                    stop=True,
                )
            if (t, u) in SCHR_UNITS:
                # Schraudolph: exp(x) ~= bitcast_fp16(round(A16*x + B16));
                # one fused DVE tensor_scalar offloads this unit from ACT.
                pti = sb.tile(
                    [128, UNIT_CAP * JT], I16, tag="pt16", bufs=2, name="pti"
                )
                nc.vector.tensor_scalar(
                    pti[:, 0 : size * JT], st[:, 0 : size * JT],
                    A16, B16, mybir.AluOpType.mult, mybir.AluOpType.add,
                )
                return pti.bitcast(F16)
            pt = sb.tile([128, UNIT_CAP * JT], BF16, tag="pt", bufs=3)
            nc.scalar.activation(
                pt[:, 0 : size * JT], st[:, 0 : size * JT],
                mybir.ActivationFunctionType.Exp,
            )
            for p, ent in enumerate(unit):
                if ent[0] == "D":
                    d = ent[1]
                    nc.vector.tensor_mul(
                        pt[:, p * JT : (p + 1) * JT],
                        pt[:, p * JT : (p + 1) * JT],
                        masks_sb[:, d * JT : (d + 1) * JT],
                    )
            return pt

        def emit_back(t, u, unit, pt):
            if u == 0:
                ot_tiles[t] = psum.tile(
                    [128, JT], F32, tag="ot", bufs=2, name="ot_ps"
                )
            ot_ps = ot_tiles[t]
            nu = len(SLOT_UNITS[t])
            for p in range(len(unit)):
                bi = BLK_OFS[t][u] + p
                nc.tensor.matmul(
                    ot_ps[0 : NE + 1, :],
                    x6v_sb[:, bi * (NE + 1) : (bi + 1) * (NE + 1)],
                    pt[:, p * JT : (p + 1) * JT],
                    start=(u == 0 and p == 0),
                    stop=(u == nu - 1 and p == len(unit) - 1),
                    skip_group_check=True,
                )
            if u == nu - 1:
                emit_epilogue(t, ot_ps)

        def emit_epilogue(t, ot_ps):
            ot_sb = sb.tile([NE + 1, JT], F32, tag="ots", bufs=2)
            nc.vector.tensor_copy(ot_sb[:], ot_ps[0 : NE + 1, :])
            o4_sb = sb.tile([128, 4 * NE], F32, tag="o", bufs=2)
            for s in range(JT // 128):
                # transpose scratch shares the "ot" psum pool (2 banks) so
                # consecutive chunks pipeline instead of serializing.
                tr_ps = psum.tile([128, JT], F32, tag="ot", bufs=2, name="tr_ps")
                nc.tensor.transpose(
                    tr_ps[:, 0 : NE + 1],
                    ot_sb[:, s * 128 : (s + 1) * 128],
                    ident_f[0 : NE + 1, 0 : NE + 1],
                )
                rec = sb.tile([128, 1], F32, tag="rec", bufs=2)
                nc.vector.reciprocal(rec[:], tr_ps[:, NE : NE + 1])
                nc.vector.tensor_scalar_mul(
                    o4_sb[:, s * NE : (s + 1) * NE], tr_ps[:, 0:NE], rec[:]
                )
            # one batched DMA per slot: dram rows t*JT + s*128 + r map to
            # o4_sb[r, s*NE:(s+1)*NE]
            nc.sync.dma_start(
                out=out[t * JT : (t + 1) * JT, :].rearrange(
                    "(s r) c -> r s c", s=4
                ),
                in_=o4_sb[:].rearrange("r (s c) -> r s c", s=4),
            )

        prev = None
        for k, (t, u, unit) in enumerate(fronts):
            pt = emit_front(t, u, unit)
            if prev is not None:
                emit_back(*prev)
            prev = (t, u, unit, pt)
        emit_back(*prev)

    nc.compile()
    return nc


def make_in_maps(x, Wk, bk, Wq, bq, Wv, bv):
    """Build the 8 per-core input dicts from the full problem inputs."""
    x = np.asarray(x, np.float32)
    Wk = np.asarray(Wk, np.float32)
    bk = np.asarray(bk, np.float32)
    Wq = np.asarray(Wq, np.float32)
    bq = np.asarray(bq, np.float32)
    Wv = np.asarray(Wv, np.float32)
    bv = np.asarray(bv, np.float32)

    # M6 folds Wk/Wq/biases and the 1/sqrt(64) score scale.
    M6 = np.zeros((6, 6), np.float32)
    M6[0:5, 0:5] = Wk @ Wq.T
    M6[0:5, 5] = Wk @ bq
    M6[5, 0:5] = Wq @ bk
    M6[5, 5] = bk @ bq
    M6 *= 0.125

    in_maps = []
    per_batch = {}
    for b in range(B):
        X6 = np.concatenate([x[:, b, :], np.ones((S, 1), np.float32)], axis=1)
        G = X6 @ M6                                   # (S, 6)
        V6 = np.concatenate(
            [x[:, b, :] @ Wv + bv[None, :], np.ones((S, 1), np.float32)], axis=1
        )                                             # (S, 65); col 64 -> den
        per_batch[b] = (X6, G, V6)

    for core in range(N_CORES):
        b, parity = core // 2, core % 2
        jos = JOS_BY_PARITY[parity]
        X6, G, V6 = per_batch[b]

        xq4 = np.zeros((128, NSLOT * JT), np.float32)
        for t in range(NSLOT):
            for grp in range(4):
                xq4[32 * grp : 32 * grp + 6, t * JT : (t + 1) * JT] = X6[
                    jos[t] : jos[t] + JT
                ].T
        g4 = np.zeros((128, N_UNITS * 128), np.float32)
        x6v = np.zeros((128, N_BLOCKS * (NE + 1)), np.float32)
        for t in range(NSLOT):
            jo = jos[t]
            nreal = jo // 128
            for u, unit in enumerate(SLOT_UNITS[t]):
                for p, ent in enumerate(unit):
                    if ent[0] == "F":
                        gb = ent[1]
                        slack = gb >= nreal
                    else:
                        gb = jo // 128 + ent[1]
                        slack = False
                    ucol = UNIT_OFS[t][u] * 128
                    g4[32 * p : 32 * p + 6, ucol : ucol + 128] = G[
                        128 * gb : 128 * gb + 128
                    ].T
                    bi = BLK_OFS[t][u] + p
                    if not slack:
                        x6v[:, bi * (NE + 1) : (bi + 1) * (NE + 1)] = V6[
                            128 * gb : 128 * gb + 128
                        ]

        in_maps.append(
            {
                "g4": np.ascontiguousarray(g4).astype(ml_dtypes.bfloat16),
                "xq4": np.ascontiguousarray(xq4).astype(ml_dtypes.bfloat16),
                "x6v": np.ascontiguousarray(x6v).astype(ml_dtypes.bfloat16),
            }
        )
    return in_maps


def assemble_output(results):
    """Stitch 8 per-core (2048, 64) outputs into (S, B, NE)."""
    out = np.zeros((S, B, NE), np.float32)
    for core in range(N_CORES):
        b, parity = core // 2, core % 2
        jos = JOS_BY_PARITY[parity]
        co = results[core]["out"]
        for t in range(NSLOT):
            out[jos[t] : jos[t] + JT, b, :] = co[t * JT : (t + 1) * JT, :]
    return out


def run_on_device(in_maps, trace=False):
    from concourse.bass_utils import run_bass_kernel_spmd

    if "nc" not in _NC_CACHE:
        _NC_CACHE["nc"] = build_graph()
    nc = _NC_CACHE["nc"]
    return run_bass_kernel_spmd(
        nc, in_maps, core_ids=list(range(N_CORES)), trace=trace
    )


def kernel(x, Wk, bk, Wq, bq, Wv, bv):
    in_maps = make_in_maps(x, Wk, bk, Wq, bq, Wv, bv)
    res = run_on_device(in_maps, trace=False)
    return assemble_output(res.results)


# revision 40
# speedup vs baseline: 1.1110x; 1.1110x over previous
"""Causal single-head attention on 8 TRN2 NeuronCores.

Problem: x:(S=4096, B=4, E=5) f32; Wk/Wq/Wv:(5,64), bk/bq/bv:(64,).
  K/Q/V = x@W + b per batch; scores = K.Q^T/8 (keys i, queries j), causal
  (key i attends query j iff i <= j), softmax over keys per query, out =
  sum_i V[i]*P[i,j] -> (S, B, 64).

Sharding: 8 cores = 4 batches x 2 query-stripe parities. Parity 0 takes
query tiles at offsets {0,1024,2048,3072}, parity 1 {512,1536,2560,3584}.
One SPMD graph; per-core differences are pure input data (parity-0 slack
key blocks have their V6 data zeroed so they add nothing to numerator or
denominator).

Key algebraic tricks (all host-precomputed):
  - scores = X6 @ M6 @ X6^T where X6 = [x | 1] (S,6) and M6 (6,6) folds
    Wk, Wq, both biases and the 1/sqrt(64) scale. G = X6 @ M6 is computed
    on host, so mm1 per 128-key block is a K=6 contraction:
    lhsT = G^T block (6,128), rhs = X6^T queries (6,512).
  - V6 = [x@Wv + bv | 1] (S,65); mm2 accumulates O^T = sum_blocks
    V6_blk^T @ P_blk directly into one PSUM bank per query slot. Column
    64 (the ones column) accumulates the softmax denominator.

Schedule per core: 4 query slots x 512 queries; per slot, key blocks
(128 keys) grouped into "triad" units of <=3 blocks. Per unit: 3
row-tiled mm1 matmuls (tile groups at partitions 0/32/64 run
concurrently on the PE), then either one Exp activation over the whole
[128, 3*512] PSUM unit (amortizes ScalarE instruction overhead) or — for
8 spread-out all-full units — a Schraudolph exp on the Vector engine
(one fused mult-add tensor_scalar into int16, bitcast to fp16), which
splits the exp work across two engines. DVE causal-mask multiplies on
diagonal blocks, then accumulating 65-row mm2 matmuls (dense PE work
also keeps the HAM clock gate at 8/8). Fronts and backs are software-
pipelined so the PE never waits on the activation of the current unit.
Slot epilogue: PE transpose + DVE reciprocal/mul + per-chunk DMA out.
An ~3.5us contiguous PE warmup burst after the first DMA chunk
un-throttles the HAM clock gate before the main loop. PSUM: st 2x3
banks + ot/warm/transpose pool 2 = 8. No max-subtraction (scores are
O(1), exp is safe in f32).
"""

import sys
from contextlib import ExitStack

import ml_dtypes
import numpy as np

for _p in ("/opt/trn_rl_repo", "/opt/pypackages"):
    if _p not in sys.path:
        sys.path.append(_p)

import concourse.bass as bass
import concourse.tile as tile
from concourse import bacc, mybir

F32 = mybir.dt.float32
F16 = mybir.dt.float16
I16 = mybir.dt.int16
BF16 = mybir.dt.bfloat16

S, B, E, NE = 4096, 4, 5, 64
N_CORES = 8
JT = 512            # query tile width
NSLOT = 4
FCNT = (4, 12, 20, 28)   # static full-block count per slot (parity max)
JOS_BY_PARITY = ((0, 1024, 2048, 3072), (512, 1536, 2560, 3584))
UNIT_CAP = 3        # key blocks per unit (3 PSUM banks per st buffer)
SLOT_ORDER = (3, 2, 1, 0)   # biggest first: shorter tail

# ---- static unit tables (parity-independent structure) ----
# entry = ('F', f_idx) or ('D', d); per slot: FCNT full blocks + 4 diag.
SLOT_UNITS = []
for _t in range(NSLOT):
    _L = [("F", g) for g in range(FCNT[_t])] + [("D", d) for d in range(4)]
    SLOT_UNITS.append([_L[i : i + UNIT_CAP] for i in range(0, len(_L), UNIT_CAP)])
N_UNITS = sum(len(u) for u in SLOT_UNITS)            # 28
N_BLOCKS = sum(len(un) for u in SLOT_UNITS for un in u)  # 80
# flat offsets in PROCESSING order (slot 3 first) so chunked DMAs can
# deliver the first-processed units' data first:
# UNIT_OFS[t][u] = unit index into g4 columns; BLK_OFS[t][u] = index of
# the unit's first block into x6v blocks.
UNIT_OFS = [[0] * len(SLOT_UNITS[_t]) for _t in range(NSLOT)]
BLK_OFS = [[0] * len(SLOT_UNITS[_t]) for _t in range(NSLOT)]
_uc, _bc = 0, 0
for _t in SLOT_ORDER:
    for _u, _un in enumerate(SLOT_UNITS[_t]):
        UNIT_OFS[_t][_u] = _uc
        BLK_OFS[_t][_u] = _bc
        _uc += 1
        _bc += len(_un)

# units whose exp runs on the vector engine via the Schraudolph fp16
# bit trick (all-F units only; spread across the timeline).
SCHR_UNITS = {(3, 2), (3, 4), (3, 6), (3, 8), (2, 2), (2, 4), (1, 2), (0, 0)}
A16 = 1024.0 / float(np.log(2.0))        # 1477.3197
B16 = 15360.0 - 45.0                     # fp16 exponent bias - spline tweak

# last unit index that contributes a block at position p (per slot), for
# mm2 PSUM accumulation stop flags.
LAST_U_AT_P = []
for _t in range(NSLOT):
    LAST_U_AT_P.append(
        [max(_u for _u, _un in enumerate(SLOT_UNITS[_t]) if len(_un) > _p)
         for _p in range(UNIT_CAP)]
    )

_NC_CACHE = {}


def build_graph():
    nc = bacc.Bacc("TRN2", target_bir_lowering=False, debug=False)

    g4 = nc.declare_dram_parameter("g4", [128, N_UNITS * 128], BF16, isOutput=False)
    xq4 = nc.declare_dram_parameter("xq4", [128, NSLOT * JT], BF16, isOutput=False)
    x6v = nc.declare_dram_parameter("x6v", [128, N_BLOCKS * (NE + 1)], BF16,
                                    isOutput=False)
    out = nc.declare_dram_parameter("out", [NSLOT * JT, NE], F32, isOutput=True)

    with tile.TileContext(nc) as tc, ExitStack() as ctx:
        consts = ctx.enter_context(tc.tile_pool(name="consts", bufs=1))
        psum = ctx.enter_context(tc.tile_pool(name="psum", bufs=2, space="PSUM"))
        sb = ctx.enter_context(tc.tile_pool(name="sb", bufs=2))

        xq4_sb = consts.tile([128, NSLOT * JT], BF16)
        g4_sb = consts.tile([128, N_UNITS * 128], BF16)
        x6v_sb = consts.tile([128, N_BLOCKS * (NE + 1)], BF16)
        # Chunked input DMAs in processing order: the first-processed
        # units' stationaries land first so mm1 of unit 0 starts ~1us in.
        t0 = SLOT_ORDER[0]
        nc.sync.dma_start(
            out=xq4_sb[:, t0 * JT : (t0 + 1) * JT],
            in_=xq4[:, t0 * JT : (t0 + 1) * JT],
        )
        G4C0, BLKC0 = 4 * 128, 12 * (NE + 1)
        nc.sync.dma_start(out=g4_sb[:, 0:G4C0], in_=g4[:, 0:G4C0])
        nc.sync.dma_start(out=x6v_sb[:, 0:BLKC0], in_=x6v[:, 0:BLKC0])
        nc.sync.dma_start(out=g4_sb[:, G4C0:], in_=g4[:, G4C0:])
        nc.sync.dma_start(out=xq4_sb[:, 0 : t0 * JT], in_=xq4[:, 0 : t0 * JT])
        nc.sync.dma_start(out=x6v_sb[:, BLKC0:], in_=x6v[:, BLKC0:])

        # PE warmup: one contiguous ~4.5us burst of matmuls right after
        # the first DMA chunk lands, covering a full free-running HAM
        # window so the PE clock un-gates to 8/8 before the main loop.
        warm_ps = psum.tile([128, JT], F32, tag="ot", bufs=2, name="warm_ps")
        for _ in range(8):
            nc.tensor.matmul(
                warm_ps[:, :],
                xq4_sb[0:6, t0 * JT : t0 * JT + 128],
                xq4_sb[0:6, t0 * JT : (t0 + 1) * JT],
                start=True, stop=True, skip_group_check=True,
            )

        # identity for PE transpose
        ident_f = consts.tile([128, 128], F32)
        from concourse.masks import make_identity

        make_identity(nc, ident_f[:])

        # diagonal causal masks, core-invariant: mask_d[p, q] = (p + 128d <= q)
        masks_f = consts.tile([128, 4 * JT], F32)
        nc.gpsimd.memset(masks_f[:], 1.0)
        for d in range(4):
            nc.gpsimd.affine_select(
                out=masks_f[:, d * JT : (d + 1) * JT],
                in_=masks_f[:, d * JT : (d + 1) * JT],
                compare_op=mybir.AluOpType.is_ge,
                fill=0.0,
                base=-128 * d,
                pattern=[[1, JT]],
                channel_multiplier=-1,
            )
        masks_sb = consts.tile([128, 4 * JT], BF16)
        nc.vector.tensor_copy(masks_sb[:], masks_f[:])

        # ---- main pipeline ----
        # front(k): mm1 unit k + Exp + masks.  back(k): mm2 accumulate
        # (+ slot epilogue when k closes a slot), emitted after front(k+1)
        # so the PE never waits on ACT(k) before starting mm1(k+1).
        fronts = []
        for t in SLOT_ORDER:
            for u, unit in enumerate(SLOT_UNITS[t]):
                fronts.append((t, u, unit))

        ot_tiles = {}

        def emit_front(t, u, unit):
            size = len(unit)
            st = psum.tile([128, UNIT_CAP * JT], F32, tag="st", bufs=2)
            gcol = UNIT_OFS[t][u] * 128
            for p in range(size):
                nc.tensor.matmul(
                    st[:, p * JT : (p + 1) * JT],
                    g4_sb[32 * p : 32 * p + 6, gcol : gcol + 128],
                    xq4_sb[32 * p : 32 * p + 6, t * JT : (t + 1) * JT],
                    start=True,
                    stop=True,
                )
            if (t, u) in SCHR_UNITS:
                # Schraudolph: exp(x) ~= bitcast_fp16(round(A16*x + B16));
                # one fused DVE tensor_scalar offloads this unit from ACT.
                pti = sb.tile(
                    [128, UNIT_CAP * JT], I16, tag="pt16", bufs=2, name="pti"
                )
                nc.vector.tensor_scalar(
                    pti[:, 0 : size * JT], st[:, 0 : size * JT],
                    A16, B16, mybir.AluOpType.mult, mybir.AluOpType.add,
                )
                return pti.bitcast(F16)
            pt = sb.tile([128, UNIT_CAP * JT], BF16, tag="pt", bufs=3)
            nc.scalar.activation(
                pt[:, 0 : size * JT], st[:, 0 : size * JT],
                mybir.ActivationFunctionType.Exp,
            )
            for p, ent in enumerate(unit):
                if ent[0] == "D":
                    d = ent[1]
                    nc.vector.tensor_mul(
                        pt[:, p * JT : (p + 1) * JT],
                        pt[:, p * JT : (p + 1) * JT],
                        masks_sb[:, d * JT : (d + 1) * JT],
                    )
            return pt

        def emit_back(t, u, unit, pt):
            if u == 0:
                ot_tiles[t] = psum.tile(
                    [128, JT], F32, tag="ot", bufs=2, name="ot_ps"
                )
            ot_ps = ot_tiles[t]
            nu = len(SLOT_UNITS[t])
            for p in range(len(unit)):
                bi = BLK_OFS[t][u] + p
                nc.tensor.matmul(
                    ot_ps[0 : NE + 1, :],
                    x6v_sb[:, bi * (NE + 1) : (bi + 1) * (NE + 1)],
                    pt[:, p * JT : (p + 1) * JT],
                    start=(u == 0 and p == 0),
                    stop=(u == nu - 1 and p == len(unit) - 1),
                    skip_group_check=True,
                )
            if u == nu - 1:
                emit_epilogue(t, ot_ps)

        def emit_epilogue(t, ot_ps):
            ot_sb = sb.tile([NE + 1, JT], F32, tag="ots", bufs=2)
            nc.vector.tensor_copy(ot_sb[:], ot_ps[0 : NE + 1, :])
            for s in range(JT // 128):
                # transpose scratch shares the "ot" psum pool (2 banks) so
                # consecutive chunks pipeline instead of serializing.
                tr_ps = psum.tile([128, JT], F32, tag="ot", bufs=2, name="tr_ps")
                nc.tensor.transpose(
                    tr_ps[:, 0 : NE + 1],
                    ot_sb[:, s * 128 : (s + 1) * 128],
                    ident_f[0 : NE + 1, 0 : NE + 1],
                )
                rec = sb.tile([128, 1], F32, tag="rec", bufs=2)
                nc.vector.reciprocal(rec[:], tr_ps[:, NE : NE + 1])
                o_sb = sb.tile([128, NE], F32, tag="o", bufs=2)
                nc.vector.tensor_scalar_mul(o_sb[:], tr_ps[:, 0:NE], rec[:])
                r0 = t * JT + s * 128
                nc.sync.dma_start(out=out[r0 : r0 + 128, :], in_=o_sb[:])

        prev = None
        for k, (t, u, unit) in enumerate(fronts):
            pt = emit_front(t, u, unit)
            if prev is not None:
                emit_back(*prev)
            prev = (t, u, unit, pt)
        emit_back(*prev)

    nc.compile()
    return nc


def make_in_maps(x, Wk, bk, Wq, bq, Wv, bv):
    """Build the 8 per-core input dicts from the full problem inputs."""
    x = np.asarray(x, np.float32)
    Wk = np.asarray(Wk, np.float32)
    bk = np.asarray(bk, np.float32)
    Wq = np.asarray(Wq, np.float32)
    bq = np.asarray(bq, np.float32)
    Wv = np.asarray(Wv, np.float32)
    bv = np.asarray(bv, np.float32)

    # M6 folds Wk/Wq/biases and the 1/sqrt(64) score scale.
    M6 = np.zeros((6, 6), np.float32)
    M6[0:5, 0:5] = Wk @ Wq.T
    M6[0:5, 5] = Wk @ bq
    M6[5, 0:5] = Wq @ bk
    M6[5, 5] = bk @ bq
    M6 *= 0.125

    in_maps = []
    per_batch = {}
    for b in range(B):
        X6 = np.concatenate([x[:, b, :], np.ones((S, 1), np.float32)], axis=1)
        G = X6 @ M6                                   # (S, 6)
        V6 = np.concatenate(
            [x[:, b, :] @ Wv + bv[None, :], np.ones((S, 1), np.float32)], axis=1
        )                                             # (S, 65); col 64 -> den
        per_batch[b] = (X6, G, V6)

    for core in range(N_CORES):
        b, parity = core // 2, core % 2
        jos = JOS_BY_PARITY[parity]
        X6, G, V6 = per_batch[b]

        xq4 = np.zeros((128, NSLOT * JT), np.float32)
        for t in range(NSLOT):
            for grp in range(4):
                xq4[32 * grp : 32 * grp + 6, t * JT : (t + 1) * JT] = X6[
                    jos[t] : jos[t] + JT
                ].T
        g4 = np.zeros((128, N_UNITS * 128), np.float32)
        x6v = np.zeros((128, N_BLOCKS * (NE + 1)), np.float32)
        for t in range(NSLOT):
            jo = jos[t]
            nreal = jo // 128
            for u, unit in enumerate(SLOT_UNITS[t]):
                for p, ent in enumerate(unit):
                    if ent[0] == "F":
                        gb = ent[1]
                        slack = gb >= nreal
                    else:
                        gb = jo // 128 + ent[1]
                        slack = False
                    ucol = UNIT_OFS[t][u] * 128
                    g4[32 * p : 32 * p + 6, ucol : ucol + 128] = G[
                        128 * gb : 128 * gb + 128
                    ].T
                    bi = BLK_OFS[t][u] + p
                    if not slack:
                        x6v[:, bi * (NE + 1) : (bi + 1) * (NE + 1)] = V6[
                            128 * gb : 128 * gb + 128
                        ]

        in_maps.append(
            {
                "g4": np.ascontiguousarray(g4).astype(ml_dtypes.bfloat16),
                "xq4": np.ascontiguousarray(xq4).astype(ml_dtypes.bfloat16),
                "x6v": np.ascontiguousarray(x6v).astype(ml_dtypes.bfloat16),
            }
        )
    return in_maps


def assemble_output(results):
    """Stitch 8 per-core (2048, 64) outputs into (S, B, NE)."""
    out = np.zeros((S, B, NE), np.float32)
    for core in range(N_CORES):
        b, parity = core // 2, core % 2
        jos = JOS_BY_PARITY[parity]
        co = results[core]["out"]
        for t in range(NSLOT):
            out[jos[t] : jos[t] + JT, b, :] = co[t * JT : (t + 1) * JT, :]
    return out


def run_on_device(in_maps, trace=False):
    from concourse.bass_utils import run_bass_kernel_spmd

    if "nc" not in _NC_CACHE:
        _NC_CACHE["nc"] = build_graph()
    nc = _NC_CACHE["nc"]
    return run_bass_kernel_spmd(
        nc, in_maps, core_ids=list(range(N_CORES)), trace=trace
    )


def kernel(x, Wk, bk, Wq, bq, Wv, bv):
    in_maps = make_in_maps(x, Wk, bk, Wq, bq, Wv, bv)
    res = run_on_device(in_maps, trace=False)
    return assemble_output(res.results)


# revision 42
# speedup vs baseline: 1.1834x; 1.0651x over previous
"""Causal single-head attention on 8 TRN2 NeuronCores.

Problem: x:(S=4096, B=4, E=5) f32; Wk/Wq/Wv:(5,64), bk/bq/bv:(64,).
  K/Q/V = x@W + b per batch; scores = K.Q^T/8 (keys i, queries j), causal
  (key i attends query j iff i <= j), softmax over keys per query, out =
  sum_i V[i]*P[i,j] -> (S, B, 64).

Sharding: 8 cores = 4 batches x 2 query-stripe parities. Parity 0 takes
query tiles at offsets {0,1024,2048,3072}, parity 1 {512,1536,2560,3584}.
One SPMD graph; per-core differences are pure input data (parity-0 slack
key blocks have their V6 data zeroed so they add nothing to numerator or
denominator).

Key algebraic tricks (all host-precomputed):
  - scores = X6 @ M6 @ X6^T where X6 = [x | 1] (S,6) and M6 (6,6) folds
    Wk, Wq, both biases and the 1/sqrt(64) scale. G = X6 @ M6 is computed
    on host, so mm1 per 128-key block is a K=6 contraction:
    lhsT = G^T block (6,128), rhs = X6^T queries (6,512).
  - V6 = [x@Wv + bv | 1] (S,65); mm2 accumulates O^T = sum_blocks
    V6_blk^T @ P_blk directly into one PSUM bank per query slot. Column
    64 (the ones column) accumulates the softmax denominator.

Schedule per core: 4 query slots x 512 queries; per slot, key blocks
(128 keys) grouped into "triad" units of <=3 blocks. Per unit: 3
row-tiled mm1 matmuls (tile groups at partitions 0/32/64 run
concurrently on the PE), then either one Exp activation over the whole
[128, 3*512] PSUM unit (amortizes ScalarE instruction overhead) or — for
8 spread-out all-full units — a Schraudolph exp on the Vector engine
(one fused mult-add tensor_scalar into int16, bitcast to fp16), which
splits the exp work across two engines. DVE causal-mask multiplies on
diagonal blocks, then accumulating 65-row mm2 matmuls (dense PE work
also keeps the HAM clock gate at 8/8). Fronts and backs are software-
pipelined so the PE never waits on the activation of the current unit.
Slot epilogue: PE transpose + DVE reciprocal/mul + per-chunk DMA out.
Input DMAs are chunked in processing order so the first unit's
stationaries land ~1us in. PSUM: st 2x3 banks + ot/transpose pool 2 = 8.
No max-subtraction (scores are O(1), exp is safe in f32).
"""

import sys
from contextlib import ExitStack

import ml_dtypes
import numpy as np

for _p in ("/opt/trn_rl_repo", "/opt/pypackages"):
    if _p not in sys.path:
        sys.path.append(_p)

import concourse.bass as bass
import concourse.tile as tile
from concourse import bacc, mybir

F32 = mybir.dt.float32
F16 = mybir.dt.float16
I16 = mybir.dt.int16
BF16 = mybir.dt.bfloat16

S, B, E, NE = 4096, 4, 5, 64
N_CORES = 8
JT = 512            # query tile width
NSLOT = 4
FCNT = (4, 12, 20, 28)   # static full-block count per slot (parity max)
JOS_BY_PARITY = ((0, 1024, 2048, 3072), (512, 1536, 2560, 3584))
UNIT_CAP = 3        # key blocks per unit (3 PSUM banks per st buffer)
SLOT_ORDER = (3, 2, 1, 0)   # biggest first: shorter tail

# ---- static unit tables (parity-independent structure) ----
# entry = ('F', f_idx) or ('D', d); per slot: FCNT full blocks + 4 diag.
SLOT_UNITS = []
for _t in range(NSLOT):
    _L = [("F", g) for g in range(FCNT[_t])] + [("D", d) for d in range(4)]
    SLOT_UNITS.append([_L[i : i + UNIT_CAP] for i in range(0, len(_L), UNIT_CAP)])
N_UNITS = sum(len(u) for u in SLOT_UNITS)            # 28
N_BLOCKS = sum(len(un) for u in SLOT_UNITS for un in u)  # 80
# flat offsets in PROCESSING order (slot 3 first) so chunked DMAs can
# deliver the first-processed units' data first:
# UNIT_OFS[t][u] = unit index into g4 columns; BLK_OFS[t][u] = index of
# the unit's first block into x6v blocks.
UNIT_OFS = [[0] * len(SLOT_UNITS[_t]) for _t in range(NSLOT)]
BLK_OFS = [[0] * len(SLOT_UNITS[_t]) for _t in range(NSLOT)]
_uc, _bc = 0, 0
for _t in SLOT_ORDER:
    for _u, _un in enumerate(SLOT_UNITS[_t]):
        UNIT_OFS[_t][_u] = _uc
        BLK_OFS[_t][_u] = _bc
        _uc += 1
        _bc += len(_un)

# units whose exp runs on the vector engine via the Schraudolph fp16
# bit trick (all-F units only; spread across the timeline).
SCHR_UNITS = {(3, 2), (3, 4), (3, 6), (3, 8), (2, 2), (2, 4), (1, 2), (0, 0)}
A16 = 1024.0 / float(np.log(2.0))        # 1477.3197
B16 = 15360.0 - 45.0                     # fp16 exponent bias - spline tweak

# last unit index that contributes a block at position p (per slot), for
# mm2 PSUM accumulation stop flags.
LAST_U_AT_P = []
for _t in range(NSLOT):
    LAST_U_AT_P.append(
        [max(_u for _u, _un in enumerate(SLOT_UNITS[_t]) if len(_un) > _p)
         for _p in range(UNIT_CAP)]
    )

_NC_CACHE = {}


def build_graph():
    nc = bacc.Bacc("TRN2", target_bir_lowering=False, debug=False)

    g4 = nc.declare_dram_parameter("g4", [128, N_UNITS * 128], BF16, isOutput=False)
    xq4 = nc.declare_dram_parameter("xq4", [128, NSLOT * JT], BF16, isOutput=False)
    x6v = nc.declare_dram_parameter("x6v", [128, N_BLOCKS * (NE + 1)], BF16,
                                    isOutput=False)
    out = nc.declare_dram_parameter("out", [NSLOT * JT, NE], F32, isOutput=True)

    with tile.TileContext(nc) as tc, ExitStack() as ctx:
        consts = ctx.enter_context(tc.tile_pool(name="consts", bufs=1))
        psum = ctx.enter_context(tc.tile_pool(name="psum", bufs=2, space="PSUM"))
        sb = ctx.enter_context(tc.tile_pool(name="sb", bufs=2))

        xq4_sb = consts.tile([128, NSLOT * JT], BF16)
        g4_sb = consts.tile([128, N_UNITS * 128], BF16)
        x6v_sb = consts.tile([128, N_BLOCKS * (NE + 1)], BF16)
        # Chunked input DMAs in processing order: the first-processed
        # units' stationaries land first so mm1 of unit 0 starts ~1us in.
        t0 = SLOT_ORDER[0]
        nc.sync.dma_start(
            out=xq4_sb[:, t0 * JT : (t0 + 1) * JT],
            in_=xq4[:, t0 * JT : (t0 + 1) * JT],
        )
        G4C0, BLKC0 = 4 * 128, 12 * (NE + 1)
        nc.sync.dma_start(out=g4_sb[:, 0:G4C0], in_=g4[:, 0:G4C0])
        nc.sync.dma_start(out=x6v_sb[:, 0:BLKC0], in_=x6v[:, 0:BLKC0])
        nc.sync.dma_start(out=g4_sb[:, G4C0:], in_=g4[:, G4C0:])
        nc.sync.dma_start(out=xq4_sb[:, 0 : t0 * JT], in_=xq4[:, 0 : t0 * JT])
        nc.sync.dma_start(out=x6v_sb[:, BLKC0:], in_=x6v[:, BLKC0:])

        # identity for PE transpose
        ident_f = consts.tile([128, 128], F32)
        from concourse.masks import make_identity

        make_identity(nc, ident_f[:])

        # diagonal causal masks, core-invariant: mask_d[p, q] = (p + 128d <= q)
        masks_f = consts.tile([128, 4 * JT], F32)
        nc.gpsimd.memset(masks_f[:], 1.0)
        for d in range(4):
            nc.gpsimd.affine_select(
                out=masks_f[:, d * JT : (d + 1) * JT],
                in_=masks_f[:, d * JT : (d + 1) * JT],
                compare_op=mybir.AluOpType.is_ge,
                fill=0.0,
                base=-128 * d,
                pattern=[[1, JT]],
                channel_multiplier=-1,
            )
        masks_sb = consts.tile([128, 4 * JT], BF16)
        nc.vector.tensor_copy(masks_sb[:], masks_f[:])

        # ---- main pipeline ----
        # front(k): mm1 unit k + Exp + masks.  back(k): mm2 accumulate
        # (+ slot epilogue when k closes a slot), emitted after front(k+1)
        # so the PE never waits on ACT(k) before starting mm1(k+1).
        fronts = []
        for t in SLOT_ORDER:
            for u, unit in enumerate(SLOT_UNITS[t]):
                fronts.append((t, u, unit))

        ot_tiles = {}

        def emit_front(t, u, unit):
            size = len(unit)
            st = psum.tile([128, UNIT_CAP * JT], F32, tag="st", bufs=2)
            gcol = UNIT_OFS[t][u] * 128
            for p in range(size):
                nc.tensor.matmul(
                    st[:, p * JT : (p + 1) * JT],
                    g4_sb[32 * p : 32 * p + 6, gcol : gcol + 128],
                    xq4_sb[32 * p : 32 * p + 6, t * JT : (t + 1) * JT],
                    start=True,
                    stop=True,
                )
            if (t, u) in SCHR_UNITS:
                # Schraudolph: exp(x) ~= bitcast_fp16(round(A16*x + B16));
                # one fused DVE tensor_scalar offloads this unit from ACT.
                pti = sb.tile(
                    [128, UNIT_CAP * JT], I16, tag="pt16", bufs=2, name="pti"
                )
                nc.vector.tensor_scalar(
                    pti[:, 0 : size * JT], st[:, 0 : size * JT],
                    A16, B16, mybir.AluOpType.mult, mybir.AluOpType.add,
                )
                return pti.bitcast(F16)
            pt = sb.tile([128, UNIT_CAP * JT], BF16, tag="pt", bufs=3)
            nc.scalar.activation(
                pt[:, 0 : size * JT], st[:, 0 : size * JT],
                mybir.ActivationFunctionType.Exp,
            )
            for p, ent in enumerate(unit):
                if ent[0] == "D":
                    d = ent[1]
                    nc.vector.tensor_mul(
                        pt[:, p * JT : (p + 1) * JT],
                        pt[:, p * JT : (p + 1) * JT],
                        masks_sb[:, d * JT : (d + 1) * JT],
                    )
            return pt

        def emit_back(t, u, unit, pt):
            if u == 0:
                ot_tiles[t] = psum.tile(
                    [128, JT], F32, tag="ot", bufs=2, name="ot_ps"
                )
            ot_ps = ot_tiles[t]
            nu = len(SLOT_UNITS[t])
            for p in range(len(unit)):
                bi = BLK_OFS[t][u] + p
                nc.tensor.matmul(
                    ot_ps[0 : NE + 1, :],
                    x6v_sb[:, bi * (NE + 1) : (bi + 1) * (NE + 1)],
                    pt[:, p * JT : (p + 1) * JT],
                    start=(u == 0 and p == 0),
                    stop=(u == nu - 1 and p == len(unit) - 1),
                    skip_group_check=True,
                )
            if u == nu - 1:
                emit_epilogue(t, ot_ps)

        def emit_epilogue(t, ot_ps):
            ot_sb = sb.tile([NE + 1, JT], F32, tag="ots", bufs=2)
            nc.vector.tensor_copy(ot_sb[:], ot_ps[0 : NE + 1, :])
            for s in range(JT // 128):
                # transpose scratch shares the "ot" psum pool (2 banks) so
                # consecutive chunks pipeline instead of serializing.
                tr_ps = psum.tile([128, JT], F32, tag="ot", bufs=2, name="tr_ps")
                nc.tensor.transpose(
                    tr_ps[:, 0 : NE + 1],
                    ot_sb[:, s * 128 : (s + 1) * 128],
                    ident_f[0 : NE + 1, 0 : NE + 1],
                )
                rec = sb.tile([128, 1], F32, tag="rec", bufs=2)
                nc.vector.reciprocal(rec[:], tr_ps[:, NE : NE + 1])
                o_sb = sb.tile([128, NE], F32, tag="o", bufs=2)
                nc.vector.tensor_scalar_mul(o_sb[:], tr_ps[:, 0:NE], rec[:])
                r0 = t * JT + s * 128
                nc.sync.dma_start(out=out[r0 : r0 + 128, :], in_=o_sb[:])

        prev = None
        for k, (t, u, unit) in enumerate(fronts):
            pt = emit_front(t, u, unit)
            if prev is not None:
                emit_back(*prev)
            prev = (t, u, unit, pt)
        emit_back(*prev)

    nc.compile()
    return nc


def make_in_maps(x, Wk, bk, Wq, bq, Wv, bv):
    """Build the 8 per-core input dicts from the full problem inputs."""
    x = np.asarray(x, np.float32)
    Wk = np.asarray(Wk, np.float32)
    bk = np.asarray(bk, np.float32)
    Wq = np.asarray(Wq, np.float32)
    bq = np.asarray(bq, np.float32)
    Wv = np.asarray(Wv, np.float32)
    bv = np.asarray(bv, np.float32)

    # M6 folds Wk/Wq/biases and the 1/sqrt(64) score scale.
    M6 = np.zeros((6, 6), np.float32)
    M6[0:5, 0:5] = Wk @ Wq.T
    M6[0:5, 5] = Wk @ bq
    M6[5, 0:5] = Wq @ bk
    M6[5, 5] = bk @ bq
    M6 *= 0.125

    in_maps = []
    per_batch = {}
    for b in range(B):
        X6 = np.concatenate([x[:, b, :], np.ones((S, 1), np.float32)], axis=1)
        G = X6 @ M6                                   # (S, 6)
        V6 = np.concatenate(
            [x[:, b, :] @ Wv + bv[None, :], np.ones((S, 1), np.float32)], axis=1
        )                                             # (S, 65); col 64 -> den
        per_batch[b] = (X6, G, V6)

    for core in range(N_CORES):
        b, parity = core // 2, core % 2
        jos = JOS_BY_PARITY[parity]
        X6, G, V6 = per_batch[b]

        xq4 = np.zeros((128, NSLOT * JT), np.float32)
        for t in range(NSLOT):
            for grp in range(4):
                xq4[32 * grp : 32 * grp + 6, t * JT : (t + 1) * JT] = X6[
                    jos[t] : jos[t] + JT
                ].T
        g4 = np.zeros((128, N_UNITS * 128), np.float32)
        x6v = np.zeros((128, N_BLOCKS * (NE + 1)), np.float32)
        for t in range(NSLOT):
            jo = jos[t]
            nreal = jo // 128
            for u, unit in enumerate(SLOT_UNITS[t]):
                for p, ent in enumerate(unit):
                    if ent[0] == "F":
                        gb = ent[1]
                        slack = gb >= nreal
                    else:
                        gb = jo // 128 + ent[1]
                        slack = False
                    ucol = UNIT_OFS[t][u] * 128
                    g4[32 * p : 32 * p + 6, ucol : ucol + 128] = G[
                        128 * gb : 128 * gb + 128
                    ].T
                    bi = BLK_OFS[t][u] + p
                    if not slack:
                        x6v[:, bi * (NE + 1) : (bi + 1) * (NE + 1)] = V6[
                            128 * gb : 128 * gb + 128
                        ]

        in_maps.append(
            {
                "g4": np.ascontiguousarray(g4).astype(ml_dtypes.bfloat16),
                "xq4": np.ascontiguousarray(xq4).astype(ml_dtypes.bfloat16),
                "x6v": np.ascontiguousarray(x6v).astype(ml_dtypes.bfloat16),
            }
        )
    return in_maps


def assemble_output(results):
    """Stitch 8 per-core (2048, 64) outputs into (S, B, NE)."""
    out = np.zeros((S, B, NE), np.float32)
    for core in range(N_CORES):
        b, parity = core // 2, core % 2
        jos = JOS_BY_PARITY[parity]
        co = results[core]["out"]
        for t in range(NSLOT):
            out[jos[t] : jos[t] + JT, b, :] = co[t * JT : (t + 1) * JT, :]
    return out


def run_on_device(in_maps, trace=False):
    from concourse.bass_utils import run_bass_kernel_spmd

    if "nc" not in _NC_CACHE:
        _NC_CACHE["nc"] = build_graph()
    nc = _NC_CACHE["nc"]
    return run_bass_kernel_spmd(
        nc, in_maps, core_ids=list(range(N_CORES)), trace=trace
    )


def kernel(x, Wk, bk, Wq, bq, Wv, bv):
    in_maps = make_in_maps(x, Wk, bk, Wq, bq, Wv, bv)
    res = run_on_device(in_maps, trace=False)
    return assemble_output(res.results)
